# revision 2
# baseline (speedup 1.0000x reference)
"""Trainium2 Bass kernel for nn_AttentionBlock (GroupNorm + single-head
self-attention over 4096 tokens + output projection + residual).

Sharding (8 cores): data-parallel over batch (2) x sequence-parallel over
the query dimension (4 shards of 1024). Each core recomputes GroupNorm
stats and full K/V for its batch (replicated; no collectives), computes
attention for its 1024 queries, and writes its [1024, 512] output shard.

Device layout choices (all picked so no on-device transposes are needed):
  - x is passed channels-major (xT [512, 4096] bf16). QKV projections then
    produce kT/qT channels-major and v sequence-major directly.
  - GroupNorm is folded into the projection weights: W' = W * scale_c and
    an effective bias; stats come from bn_stats on xT + tiny mask matmuls.
  - scores are computed transposed (scoresT [kpos, q]) so the softmax sum
    over kpos is a ones-vector matmul and attn@v / (attn@v)@Wp chain flows
    without transposes.
  - softmax skips max-subtraction: scores * 1/sqrt(512) stay within ~+-2
    for this problem family, exactly representable range for exp in fp32.
  - K-projection bias is dropped entirely: softmax over keys is invariant
    to it (it shifts every score in a row by the same amount).
"""

import math
import sys

import numpy as np

for _p in ("/opt/trn_rl_repo",):
    if _p not in sys.path:
        sys.path.append(_p)

import ml_dtypes  # noqa: E402

import concourse.bacc as bacc  # noqa: E402
import concourse.tile as tile  # noqa: E402
from concourse import mybir  # noqa: E402
from concourse.bass_utils import run_bass_kernel_spmd  # noqa: E402

B, H, W_, C = 2, 64, 64, 512
S = H * W_            # 4096 sequence length
NSHARD = 4            # query shards per batch
SQ = S // NSHARD      # 1024 queries per core
G = 32                # groups
GS = C // G           # 16 channels per group
EPS = 1e-5
P = 128
CCH = C // P          # 4 channel chunks of 128
NB = 512              # matmul moving free-dim block (one PSUM bank of fp32)
KCH = S // P          # 32 key chunks of 128
SM_SCALE = 1.0 / math.sqrt(C)

F32 = mybir.dt.float32
BF16 = mybir.dt.bfloat16
AL = mybir.AluOpType
AF = mybir.ActivationFunctionType
BF16_NP = ml_dtypes.bfloat16


def build_program():
    nc = bacc.Bacc(trn_type="TRN2", target_bir_lowering=False, debug=False,
                   enable_asserts=False, num_devices=8)
    d = {}

    def din(name, shape, dt):
        d[name] = nc.dram_tensor(name, list(shape), dt, kind="ExternalInput").ap()

    din("xT", (C, S), BF16)        # full batch, channels-major
    din("xqT", (C, SQ), BF16)      # this core's query columns, channels-major
    din("xq", (SQ, C), F32)        # residual rows (+ bp already added on host)
    din("Wq", (C, C), F32)
    din("Wk", (C, C), F32)
    din("Wv", (C, C), F32)
    din("Wp", (C, C), BF16)
    din("gcol", (P, CCH), F32)     # gamma, column layout: [p, cc] = gamma[cc*128+p]
    din("bcol", (P, CCH), F32)     # beta
    din("bqcol", (P, CCH), F32)    # bq
    din("bvrow", (1, C), F32)      # bv
    din("mask16", (C, G), F32)     # [c, g] = (c//16 == g) / 16
    din("maskT", (G, C), F32)      # [g, c] = (c//16 == g)
    y = nc.dram_tensor("y", [SQ, C], F32, kind="ExternalOutput").ap()
    y3 = y.rearrange("(q p) c -> p q c", p=P)

    with tile.TileContext(nc) as tc:
        with tc.tile_pool(name="persist", bufs=1) as persist, \
             tc.tile_pool(name="work", bufs=2) as work:

            # ---------------- loads ----------------
            xT = persist.tile([P, CCH, S], BF16, tag="xT")
            for cc in range(CCH):
                nc.sync.dma_start(out=xT[:, cc, :], in_=d["xT"][cc * P:(cc + 1) * P, :])
            xqT = persist.tile([P, CCH, SQ], BF16, tag="xqT")
            for cc in range(CCH):
                nc.sync.dma_start(out=xqT[:, cc, :], in_=d["xqT"][cc * P:(cc + 1) * P, :])
            xq = persist.tile([P, SQ // P, C], F32, tag="xq")
            nc.sync.dma_start(out=xq, in_=d["xq"].rearrange("(q p) c -> p q c", p=P))
            wp = persist.tile([P, CCH, C], BF16, tag="Wp")
            nc.sync.dma_start(out=wp, in_=d["Wp"].rearrange("(cc p) o -> p cc o", p=P))
            smalls = {}
            for nm in ("gcol", "bcol", "bqcol", "bvrow", "maskT"):
                smalls[nm] = persist.tile(list(d[nm].shape), F32, tag=nm, name=nm + "_sb")
                nc.sync.dma_start(out=smalls[nm], in_=d[nm])
            mask16 = persist.tile([P, CCH, G], F32, tag="mask16")
            nc.sync.dma_start(out=mask16, in_=d["mask16"].rearrange("(cc p) g -> p cc g", p=P))
            ones_bf = persist.tile([P, 1], BF16, tag="ones")
            nc.vector.memset(ones_bf, 1.0)
            eps_t = persist.tile([G, 1], F32, tag="eps")
            nc.vector.memset(eps_t, EPS)

            wb = {}   # folded bf16 weights
            bqe = persist.tile([P, CCH], F32, tag="bqe")
            bvbc = persist.tile([P, C], F32, tag="bvbc")

            with tc.tile_pool(name="wts", bufs=1) as wtsp, \
                 tc.tile_pool(name="psA", bufs=2, space="PSUM") as psA:
                wts = {}
                for wnm in ("Wq", "Wk", "Wv"):
                    wts[wnm] = wtsp.tile([P, CCH, C], F32, tag=wnm, name=wnm + "_sb")
                    nc.sync.dma_start(out=wts[wnm],
                                      in_=d[wnm].rearrange("(cc p) o -> p cc o", p=P))

                # ---------------- GroupNorm stats ----------------
                # per-channel mean / E[x^2] over the 4096 positions
                stat2 = work.tile([P, CCH, 2], F32, tag="stat2")
                for cc in range(CCH):
                    bns = work.tile([P, 8, 6], F32, tag="bns")
                    for nsub in range(8):
                        nc.vector.bn_stats(out=bns[:, nsub, :],
                                           in_=xT[:, cc, nsub * 512:(nsub + 1) * 512])
                    mv = work.tile([P, 2], F32, tag="mv")
                    nc.vector.bn_aggr(out=mv, in_=bns)
                    nc.vector.tensor_copy(stat2[:, cc, 0:1], mv[:, 0:1])
                    # E[x^2] = mu^2 + var
                    nc.vector.scalar_tensor_tensor(
                        out=stat2[:, cc, 1:2], in0=mv[:, 0:1], scalar=mv[:, 0:1],
                        in1=mv[:, 1:2], op0=AL.mult, op1=AL.add)

                # group means: [32, 2] = sum_c mask16[c, g] * stat2[c, :]
                gstat_ps = psA.tile([G, 2], F32, tag="small")
                for cc in range(CCH):
                    nc.tensor.matmul(gstat_ps, lhsT=mask16[:, cc, :], rhs=stat2[:, cc, :],
                                     start=(cc == 0), stop=(cc == CCH - 1))
                mvg = work.tile([G, 2], F32, tag="mvg")
                nc.vector.tensor_copy(mvg, gstat_ps)
                # -var = mu_g^2 - E2_g ; rstd = 1/sqrt(var + eps)
                nvar = work.tile([G, 1], F32, tag="nvar")
                nc.vector.scalar_tensor_tensor(out=nvar, in0=mvg[:, 0:1], scalar=mvg[:, 0:1],
                                               in1=mvg[:, 1:2], op0=AL.mult, op1=AL.subtract)
                sq = work.tile([G, 1], F32, tag="sq")
                nc.scalar.activation(out=sq, in_=nvar, func=AF.Sqrt, bias=eps_t, scale=-1.0)
                gb = work.tile([G, 2], F32, tag="gb")
                nc.vector.reciprocal(out=gb[:, 0:1], in_=sq)
                nc.vector.tensor_mul(gb[:, 1:2], mvg[:, 0:1], gb[:, 0:1])

                # expand to per-channel rstd / mu*rstd, then scale/shift
                sc = work.tile([P, CCH], F32, tag="sc")
                sh = work.tile([P, CCH], F32, tag="sh")
                for cc in range(CCH):
                    e_ps = psA.tile([P, 2], F32, tag="small")
                    nc.tensor.matmul(e_ps, lhsT=smalls["maskT"][:, cc * P:(cc + 1) * P],
                                     rhs=gb, start=True, stop=True)
                    rc = work.tile([P, 2], F32, tag="rc")
                    nc.vector.tensor_copy(rc, e_ps)
                    nc.vector.tensor_mul(sc[:, cc:cc + 1], rc[:, 0:1], smalls["gcol"][:, cc:cc + 1])
                    tmp = work.tile([P, 1], F32, tag="tmpsh")
                    nc.vector.tensor_mul(tmp, rc[:, 1:2], smalls["gcol"][:, cc:cc + 1])
                    nc.vector.scalar_tensor_tensor(out=sh[:, cc:cc + 1], in0=tmp, scalar=-1.0,
                                                   in1=smalls["bcol"][:, cc:cc + 1],
                                                   op0=AL.mult, op1=AL.add)

                # ---------------- fold GroupNorm into weights ----------------
                for wnm in ("Wq", "Wk", "Wv"):
                    wb[wnm] = persist.tile([P, CCH, C], BF16, tag=wnm + "b", name=wnm + "_fold")
                    for cc in range(CCH):
                        nc.vector.tensor_scalar_mul(out=wb[wnm][:, cc, :],
                                                    in0=wts[wnm][:, cc, :],
                                                    scalar1=sc[:, cc:cc + 1])
                # effective q bias: bq + Wq^T @ shift  (k bias is softmax-invariant)
                for oc in range(CCH):
                    b_ps = psA.tile([P, 1], F32, tag="small")
                    for cc in range(CCH):
                        nc.tensor.matmul(b_ps, lhsT=wts["Wq"][:, cc, oc * P:(oc + 1) * P],
                                         rhs=sh[:, cc:cc + 1],
                                         start=(cc == 0), stop=(cc == CCH - 1))
                    nc.vector.tensor_add(bqe[:, oc:oc + 1], b_ps, smalls["bqcol"][:, oc:oc + 1])
                # effective v bias row: bv + Wv^T @ shift, broadcast to 128 partitions
                bv_ps = psA.tile([1, C], F32, tag="small")
                for cc in range(CCH):
                    nc.tensor.matmul(bv_ps, lhsT=sh[:, cc:cc + 1], rhs=wts["Wv"][:, cc, :],
                                     start=(cc == 0), stop=(cc == CCH - 1))
                bve = work.tile([1, C], F32, tag="bve")
                nc.vector.tensor_add(bve, bv_ps, smalls["bvrow"])
                nc.gpsimd.partition_broadcast(bvbc, bve)

            # ---------------- QKV projections ----------------
            kT = persist.tile([P, CCH, S], BF16, tag="kT")
            v = persist.tile([P, KCH, C], BF16, tag="v")
            qT = persist.tile([P, CCH, SQ], BF16, tag="qT")
            with tc.tile_pool(name="psmm", bufs=4, space="PSUM") as psmm:
                for oc in range(CCH):
                    for nb in range(S // NB):
                        m_ps = psmm.tile([P, NB], F32, tag="mm")
                        for cc in range(CCH):
                            nc.tensor.matmul(m_ps, lhsT=wb["Wk"][:, cc, oc * P:(oc + 1) * P],
                                             rhs=xT[:, cc, nb * NB:(nb + 1) * NB],
                                             start=(cc == 0), stop=(cc == CCH - 1))
                        nc.scalar.copy(out=kT[:, oc, nb * NB:(nb + 1) * NB], in_=m_ps)
                for sb in range(KCH):
                    m_ps = psmm.tile([P, C], F32, tag="mm")
                    for cc in range(CCH):
                        nc.tensor.matmul(m_ps, lhsT=xT[:, cc, sb * P:(sb + 1) * P],
                                         rhs=wb["Wv"][:, cc, :],
                                         start=(cc == 0), stop=(cc == CCH - 1))
                    nc.vector.tensor_add(v[:, sb, :], m_ps, bvbc)
                for oc in range(CCH):
                    for qb in range(SQ // NB):
                        m_ps = psmm.tile([P, NB], F32, tag="mm")
                        for cc in range(CCH):
                            nc.tensor.matmul(m_ps, lhsT=wb["Wq"][:, cc, oc * P:(oc + 1) * P],
                                             rhs=xqT[:, cc, qb * NB:(qb + 1) * NB],
                                             start=(cc == 0), stop=(cc == CCH - 1))
                        nc.vector.tensor_scalar_add(out=qT[:, oc, qb * NB:(qb + 1) * NB],
                                                    in0=m_ps, scalar1=bqe[:, oc:oc + 1])

            # ---------------- attention ----------------
            with tc.tile_pool(name="ps_s", bufs=2, space="PSUM") as ps_s, \
                 tc.tile_pool(name="ps_o", bufs=4, space="PSUM") as ps_o, \
                 tc.tile_pool(name="ps_cs", bufs=1, space="PSUM") as ps_cs, \
                 tc.tile_pool(name="ps_y", bufs=1, space="PSUM") as ps_y, \
                 tc.tile_pool(name="ptp", bufs=4) as ptp, \
                 tc.tile_pool(name="otp", bufs=2) as otp, \
                 tc.tile_pool(name="ytp", bufs=2) as ytp, \
                 tc.tile_pool(name="sml", bufs=2) as sml:
                for qb in range(SQ // NB):
                    o_ps = [ps_o.tile([P, NB], F32, tag="o", name=f"o_ps{_cc}") for _cc in range(CCH)]
                    cs_ps = ps_cs.tile([1, NB], F32, tag="cs")
                    for kc in range(KCH):
                        s_ps = ps_s.tile([P, NB], F32, tag="s")
                        for cc in range(CCH):
                            nc.tensor.matmul(s_ps, lhsT=kT[:, cc, kc * P:(kc + 1) * P],
                                             rhs=qT[:, cc, qb * NB:(qb + 1) * NB],
                                             start=(cc == 0), stop=(cc == CCH - 1))
                        pt = ptp.tile([P, NB], BF16, tag="pt")
                        nc.scalar.activation(out=pt, in_=s_ps, func=AF.Exp, scale=SM_SCALE)
                        nc.tensor.matmul(cs_ps, lhsT=ones_bf, rhs=pt,
                                         start=(kc == 0), stop=(kc == KCH - 1))
                        for cc in range(CCH):
                            nc.tensor.matmul(o_ps[cc], lhsT=v[:, kc, cc * P:(cc + 1) * P],
                                             rhs=pt, start=(kc == 0), stop=(kc == KCH - 1))
                    csr = sml.tile([1, NB], F32, tag="csr")
                    nc.vector.reciprocal(out=csr, in_=cs_ps)
                    rbc = sml.tile([P, NB], F32, tag="rbc")
                    nc.gpsimd.partition_broadcast(rbc, csr)
                    oT = otp.tile([P, CCH, NB], BF16, tag="oT")
                    for cc in range(CCH):
                        nc.vector.tensor_mul(oT[:, cc, :], o_ps[cc], rbc)
                    for ms in range(NB // P):
                        y_ps = ps_y.tile([P, C], F32, tag="y")
                        for cc in range(CCH):
                            nc.tensor.matmul(y_ps, lhsT=oT[:, cc, ms * P:(ms + 1) * P],
                                             rhs=wp[:, cc, :],
                                             start=(cc == 0), stop=(cc == CCH - 1))
                        qi = qb * (NB // P) + ms
                        y_sb = ytp.tile([P, C], F32, tag="ysb")
                        nc.vector.tensor_add(y_sb, y_ps, xq[:, qi, :])
                        nc.sync.dma_start(out=y3[:, qi, :], in_=y_sb)
    nc.compile()
    return nc


_PROG = None


def _get_prog():
    global _PROG
    if _PROG is None:
        _PROG = build_program()
    return _PROG


def make_in_maps(inputs, gamma, beta, Wq, bq, Wk, bk, Wv, bv, Wp, bp):
    x = np.asarray(inputs, np.float32).reshape(B, S, C)
    gamma = np.asarray(gamma, np.float32)
    beta = np.asarray(beta, np.float32)
    Wq = np.ascontiguousarray(np.asarray(Wq, np.float32))
    Wk = np.ascontiguousarray(np.asarray(Wk, np.float32))
    Wv = np.ascontiguousarray(np.asarray(Wv, np.float32))
    Wp_bf = np.asarray(Wp, np.float32).astype(BF16_NP)
    bq = np.asarray(bq, np.float32)
    bv = np.asarray(bv, np.float32)
    bp = np.asarray(bp, np.float32)

    def col(vec):
        return np.ascontiguousarray(vec.reshape(CCH, P).T)

    mask16 = np.zeros((C, G), np.float32)
    mask16[np.arange(C), np.arange(C) // GS] = 1.0 / GS
    maskT = np.ascontiguousarray((mask16.T > 0).astype(np.float32) * 1.0)

    shared = {
        "Wq": Wq, "Wk": Wk, "Wv": Wv, "Wp": Wp_bf,
        "gcol": col(gamma), "bcol": col(beta), "bqcol": col(bq),
        "bvrow": np.ascontiguousarray(bv.reshape(1, C)),
        "mask16": mask16, "maskT": maskT,
    }
    in_maps = []
    for b in range(B):
        xT_b = np.ascontiguousarray(x[b].T).astype(BF16_NP)
        for s_ in range(NSHARD):
            xsh = x[b, s_ * SQ:(s_ + 1) * SQ]
            in_maps.append(dict(
                shared,
                xT=xT_b,
                xqT=np.ascontiguousarray(xsh.T).astype(BF16_NP),
                xq=np.ascontiguousarray(xsh + bp[None, :]),
            ))
    return in_maps


def gather_out(results):
    outs = [r["y"] for r in results]
    yfull = np.stack([np.concatenate(outs[b * NSHARD:(b + 1) * NSHARD], axis=0)
                      for b in range(B)])
    return np.ascontiguousarray(yfull.reshape(B, H, W_, C).astype(np.float32))


def kernel(**inputs) -> np.ndarray:
    in_maps = make_in_maps(**inputs)
    nc = _get_prog()
    res = run_bass_kernel_spmd(nc, in_maps, core_ids=list(range(8)))
    return gather_out(res.results)
